# revision 7
# baseline (speedup 1.0000x reference)
"""Causal multi-head attention (16 heads, d_model 1024, seq 4096) on 8 trn2 cores.

Sharding: tensor-parallel over heads — each core owns 2 heads (a 128-wide
slice of the QKV projections and the matching 128-row slice of Wo) and
produces a partial output; the host sums the 8 partials and adds the bias.

Per-core kernel (SPMD, inputs differ per core):
  - qT/kT per head via matmul(lhsT=W_chunk, rhs=xT_chunk) -> [64, n] layout
  - v via matmul(lhsT=xT_chunk, rhs=Wv_chunk) -> [n, 64] tiles with a ones
    column appended so the attention-weight row sums come out of the same
    matmul that computes ctx (softmax denominator for free)
  - scores^T = kT-tile.T @ qT-chunk -> PSUM [128 j, 2*512 q] (two heads side
    by side on the free axis), exp on ACT straight out of PSUM, causal mask
    via affine_select (fill 0 post-exp)
  - ctx^T accumulated in PSUM over j-tiles; normalized by the ones-row sums
    via a K=1 outer-product broadcast matmul + one DVE multiply
  - out chunk = ctx^T-as-lhsT @ Wo-slice, DMA'd PSUM->DRAM
"""

import numpy as np

import concourse.bacc as bacc
import concourse.mybir as mybir
import concourse.tile as tile
from concourse.bass_utils import run_bass_kernel_spmd

P = 128
N = 4096
D = 1024
HD = 64  # head dim
KC = 8  # k chunks of 128 over D
G = 8  # n/q chunks of 512
QC = 512  # q chunk width
NJ = 32  # j tiles of 128
F32 = mybir.dt.float32
EXP = mybir.ActivationFunctionType.Exp
MULT = mybir.AluOpType.mult
IS_GE = mybir.AluOpType.is_ge

_CACHE = {}


def build():
    nc = bacc.Bacc("TRN2", target_bir_lowering=False, debug=False, num_devices=8)

    xT = nc.dram_tensor("xT", [D, N], F32, kind="ExternalInput")
    wq = nc.dram_tensor("wq", [P, D], F32, kind="ExternalInput")  # [p, kc*128+d]
    wk = nc.dram_tensor("wk", [P, D], F32, kind="ExternalInput")
    wv = nc.dram_tensor("wv", [P, D], F32, kind="ExternalInput")
    wo = nc.dram_tensor("wo", [P, D], F32, kind="ExternalInput")  # rows = core's d slice
    out = nc.dram_tensor("out", [N, D], F32, kind="ExternalOutput")

    xT_t = xT.rearrange("(kc p) n -> kc p n", p=P)

    with tile.TileContext(nc) as tc:
        with (
            tc.tile_pool(name="const", bufs=1) as const,
            tc.tile_pool(name="persist", bufs=1) as persist,
            tc.tile_pool(name="xt", bufs=16) as xtp,
            tc.tile_pool(name="wt", bufs=3) as wtp,
            tc.tile_pool(name="ctxs", bufs=2) as ctxsp,
            tc.tile_pool(name="outs", bufs=3) as outsp,
            tc.tile_pool(name="small", bufs=4) as small,
            tc.tile_pool(name="pproj", bufs=2, space="PSUM") as pproj,
            tc.tile_pool(name="psc", bufs=2, space="PSUM") as psc,
            tc.tile_pool(name="pctx", bufs=2, space="PSUM") as pctx,
        ):
            # ---- constants / persistent state ----
            wq_s = const.tile([P, D], F32, tag="wq", name="wq")
            nc.sync.dma_start(wq_s[:], wq[:])
            wk_s = const.tile([P, D], F32, tag="wk", name="wk")
            nc.sync.dma_start(wk_s[:], wk[:])
            wv_s = const.tile([P, D], F32, tag="wv", name="wv")
            nc.sync.dma_start(wv_s[:], wv[:])
            wo_s = const.tile([P, D], F32, tag="wo", name="wo")
            nc.sync.dma_start(wo_s[:], wo[:])
            ones64 = const.tile([1, HD], F32, tag="ones64", name="ones64")
            nc.vector.memset(ones64[:], 1.0)

            # per-chunk qT/kT tiles [64, 512] per head; v tiles [128, 65] per j-tile
            qT = [[persist.tile([HD, QC], F32, tag=f"qT{h}_{g}", name=f"qT{h}_{g}") for g in range(G)] for h in range(2)]
            kT = [[persist.tile([HD, QC], F32, tag=f"kT{h}_{g}", name=f"kT{h}_{g}") for g in range(G)] for h in range(2)]
            vt = [[persist.tile([P, HD + 1], F32, tag=f"vt{h}_{j}", name=f"vt{h}_{j}") for j in range(NJ)] for h in range(2)]
            for h in range(2):
                for j in range(NJ):
                    nc.gpsimd.memset(vt[h][j][:, HD : HD + 1], 1.0)

            # ---- projection of n-chunk g: fills qT/kT[*][g], vt[*][4g..4g+3] ----
            def proj_units(g):
                n0 = g * QC
                xt = []
                for kc in range(KC):
                    t = xtp.tile([P, QC], F32, tag="xt", name="xt")
                    nc.sync.dma_start(t[:], xT_t[kc, :, n0 : n0 + QC])
                    xt.append(t)
                for which, (w_s, dst) in enumerate(((wq_s, qT), (wk_s, kT))):
                    ps = pproj.tile([P, QC], F32, tag="pp", name="pp")
                    for kc in range(KC):
                        nc.tensor.matmul(
                            ps[:], w_s[:, kc * P : (kc + 1) * P], xt[kc][:],
                            start=(kc == 0), stop=(kc == KC - 1),
                        )
                    nc.vector.tensor_copy(dst[0][g][:], ps[0:HD, :])
                    nc.vector.tensor_copy(dst[1][g][:], ps[HD:P, :])
                    yield
                for s in range(4):
                    jt = g * 4 + s
                    ps = pproj.tile([P, QC], F32, tag="pp", name="pp")
                    for kc in range(KC):
                        nc.tensor.matmul(
                            ps[:, 0:P], xt[kc][:, s * P : (s + 1) * P],
                            wv_s[:, kc * P : (kc + 1) * P],
                            start=(kc == 0), stop=(kc == KC - 1),
                        )
                    nc.vector.tensor_copy(vt[0][jt][:, 0:HD], ps[:, 0:HD])
                    nc.vector.tensor_copy(vt[1][jt][:, 0:HD], ps[:, HD:P])
                    yield

            # ---- attention for q-chunk g (proj work for g+1 interleaved) ----
            def attention(g, interleave):
                q0 = g * QC
                njt = 4 * (g + 1)
                ctx = [pctx.tile([HD + 1, QC], F32, tag="ctx", name="ctx") for h in range(2)]
                pend = None
                for jt in range(njt):
                    sc = psc.tile([P, 2 * QC], F32, tag="sc", name="sc")
                    for h in range(2):
                        nc.tensor.matmul(
                            sc[:, h * QC : (h + 1) * QC],
                            kT[h][jt // 4][:, (jt % 4) * P : (jt % 4 + 1) * P],
                            qT[h][g][:], start=True, stop=True,
                        )
                    wt = wtp.tile([P, 2 * QC], F32, tag="wt", name="wt")
                    nc.scalar.activation(wt[:], sc[:], EXP, scale=0.125)
                    if jt >= 4 * g:  # diagonal: zero where q_global < j_global
                        for h in range(2):
                            nc.gpsimd.affine_select(
                                out=wt[:, h * QC : (h + 1) * QC],
                                in_=wt[:, h * QC : (h + 1) * QC],
                                pattern=[[1, QC]], compare_op=IS_GE, fill=0.0,
                                base=q0 - jt * P, channel_multiplier=-1,
                            )
                    if pend is not None:
                        _av(*pend)
                    pend = (ctx, wt, jt, njt)
                    if interleave is not None:
                        next(interleave, None)
                _av(*pend)
                if interleave is not None:
                    for _ in interleave:
                        pass
                return ctx

            def _av(ctx, wt, jt, njt):
                for h in range(2):
                    nc.tensor.matmul(
                        ctx[h][:], vt[h][jt][:], wt[:, h * QC : (h + 1) * QC],
                        start=(jt == 0), stop=(jt == njt - 1),
                    )

            # ---- normalize + output projection for q-chunk g ----
            def finish(g, ctx):
                q0 = g * QC
                ctxs = ctxsp.tile([P, QC], F32, tag="ctxs", name="ctxs")
                for h in range(2):
                    rec = small.tile([1, QC], F32, tag="rec", name="rec")
                    nc.vector.tensor_copy(rec[:], ctx[h][HD : HD + 1, :])
                    nc.vector.reciprocal(rec[:], rec[:])
                    bc = pproj.tile([HD, QC], F32, tag="pp", name="pp")
                    nc.tensor.matmul(bc[:], ones64[:], rec[:], start=True, stop=True)
                    bcs = small.tile([HD, QC], F32, tag="bcs", name="bcs")
                    nc.vector.tensor_copy(bcs[:], bc[:])
                    nc.vector.tensor_tensor(
                        out=ctxs[h * HD : (h + 1) * HD, :],
                        in0=ctx[h][0:HD, :], in1=bcs[:], op=MULT,
                    )
                for s in range(4):
                    for oc in range(2):
                        po = pproj.tile([P, QC], F32, tag="pp", name="pp")
                        nc.tensor.matmul(
                            po[:], ctxs[:, s * P : (s + 1) * P],
                            wo_s[:, oc * QC : (oc + 1) * QC],
                            start=True, stop=True,
                        )
                        ob = outsp.tile([P, QC], F32, tag="ob", name="ob")
                        nc.vector.tensor_copy(ob[:], po[:])
                        nc.sync.dma_start(
                            out[q0 + s * P : q0 + (s + 1) * P, oc * QC : (oc + 1) * QC],
                            ob[:],
                        )

            # ---- schedule: proj(0); then attn(g) with proj(g+1) interleaved ----
            for _ in proj_units(0):
                pass
            for g in range(G):
                inter = proj_units(g + 1) if g + 1 < G else None
                ctx = attention(g, inter)
                finish(g, ctx)

    nc.compile()
    return nc


def _get_nc():
    if "nc" not in _CACHE:
        _CACHE["nc"] = build()
    return _CACHE["nc"]


def make_in_maps(x, Wq, Wk, Wv, Wo):
    x = np.asarray(x, dtype=np.float32)
    xT = np.ascontiguousarray(x.reshape(N, D).T)
    maps = []
    for c in range(8):
        sl = slice(c * P, (c + 1) * P)

        def _w(W):
            # [1024, 128] slice -> [p, kc*128+d] so SBUF gets lhsT chunks directly
            Wc = np.asarray(W, dtype=np.float32)[:, sl]
            return np.ascontiguousarray(
                Wc.reshape(KC, P, P).transpose(1, 0, 2).reshape(P, D)
            )

        maps.append(
            {
                "xT": xT,
                "wq": _w(Wq),
                "wk": _w(Wk),
                "wv": _w(Wv),
                "wo": np.ascontiguousarray(np.asarray(Wo, dtype=np.float32)[sl, :]),
            }
        )
    return maps


def kernel(x, Wq, Wk, Wv, Wo, bo):
    nc = _get_nc()
    maps = make_in_maps(x, Wq, Wk, Wv, Wo)
    res = run_bass_kernel_spmd(nc, maps, list(range(8))).results
    acc = res[0]["out"].astype(np.float32)
    for c in range(1, 8):
        acc = acc + res[c]["out"]
    acc = acc + np.asarray(bo, dtype=np.float32)[None, :]
    return acc[None].astype(np.float32)


# revision 10
# speedup vs baseline: 525.1427x; 525.1427x over previous
"""Causal multi-head attention (16 heads, d_model 1024, seq 4096) on 8 trn2 cores.

Sharding: tensor-parallel over heads — each core owns 2 heads (a 128-wide
slice of the QKV projections and the matching 128-row slice of Wo) and
produces a partial output; the host sums the 8 partials and adds the bias.

Per-core kernel (SPMD, inputs differ per core):
  - qT/kT per head via matmul(lhsT=W_chunk, rhs=xT_chunk) -> [64, n] layout
  - v via matmul(lhsT=xT_chunk, rhs=Wv_chunk) -> [n, 64] tiles with a ones
    column appended so the attention-weight row sums come out of the same
    matmul that computes ctx (softmax denominator for free)
  - scores^T = kT-tile.T @ qT-chunk -> PSUM [128 j, 2*512 q] (two heads side
    by side on the free axis), exp on ACT straight out of PSUM, causal mask
    via affine_select (fill 0 post-exp)
  - ctx^T accumulated in PSUM over j-tiles; normalized by the ones-row sums
    via a K=1 outer-product broadcast matmul + one DVE multiply
  - out chunk = ctx^T-as-lhsT @ Wo-slice, DMA'd PSUM->DRAM
"""

import jax
import numpy as np
from jax.experimental.shard_map import shard_map
from jax.sharding import Mesh, NamedSharding, PartitionSpec

import concourse.bacc as bacc
import concourse.mybir as mybir
import concourse.tile as tile

P = 128
N = 4096
D = 1024
HD = 64  # head dim
KC = 8  # k chunks of 128 over D
G = 8  # n/q chunks of 512
QC = 512  # q chunk width
NJ = 32  # j tiles of 128
F32 = mybir.dt.float32
EXP = mybir.ActivationFunctionType.Exp
MULT = mybir.AluOpType.mult
IS_GE = mybir.AluOpType.is_ge

_CACHE = {}


def build():
    nc = bacc.Bacc("TRN2", target_bir_lowering=False, debug=False, num_devices=8)

    xT = nc.dram_tensor("xT", [D, N], F32, kind="ExternalInput")
    wq = nc.dram_tensor("wq", [P, D], F32, kind="ExternalInput")  # [p, kc*128+d]
    wk = nc.dram_tensor("wk", [P, D], F32, kind="ExternalInput")
    wv = nc.dram_tensor("wv", [P, D], F32, kind="ExternalInput")
    wo = nc.dram_tensor("wo", [P, D], F32, kind="ExternalInput")  # rows = core's d slice
    out = nc.dram_tensor("out", [N, D], F32, kind="ExternalOutput")

    xT_t = xT.rearrange("(kc p) n -> kc p n", p=P)

    with tile.TileContext(nc) as tc:
        with (
            tc.tile_pool(name="const", bufs=1) as const,
            tc.tile_pool(name="persist", bufs=1) as persist,
            tc.tile_pool(name="xt", bufs=16) as xtp,
            tc.tile_pool(name="wt", bufs=3) as wtp,
            tc.tile_pool(name="ctxs", bufs=2) as ctxsp,
            tc.tile_pool(name="outs", bufs=3) as outsp,
            tc.tile_pool(name="small", bufs=4) as small,
            tc.tile_pool(name="pproj", bufs=2, space="PSUM") as pproj,
            tc.tile_pool(name="psc", bufs=2, space="PSUM") as psc,
            tc.tile_pool(name="pctx", bufs=2, space="PSUM") as pctx,
        ):
            # ---- constants / persistent state ----
            wq_s = const.tile([P, D], F32, tag="wq", name="wq")
            nc.sync.dma_start(wq_s[:], wq[:])
            wk_s = const.tile([P, D], F32, tag="wk", name="wk")
            nc.sync.dma_start(wk_s[:], wk[:])
            wv_s = const.tile([P, D], F32, tag="wv", name="wv")
            nc.sync.dma_start(wv_s[:], wv[:])
            wo_s = const.tile([P, D], F32, tag="wo", name="wo")
            nc.sync.dma_start(wo_s[:], wo[:])
            ones64 = const.tile([1, HD], F32, tag="ones64", name="ones64")
            nc.vector.memset(ones64[:], 1.0)

            # per-chunk qT/kT tiles [64, 512] per head; v tiles [128, 65] per j-tile
            qT = [[persist.tile([HD, QC], F32, tag=f"qT{h}_{g}", name=f"qT{h}_{g}") for g in range(G)] for h in range(2)]
            kT = [[persist.tile([HD, QC], F32, tag=f"kT{h}_{g}", name=f"kT{h}_{g}") for g in range(G)] for h in range(2)]
            vt = [[persist.tile([P, HD + 1], F32, tag=f"vt{h}_{j}", name=f"vt{h}_{j}") for j in range(NJ)] for h in range(2)]
            for h in range(2):
                for j in range(NJ):
                    nc.gpsimd.memset(vt[h][j][:, HD : HD + 1], 1.0)

            # ---- projection of n-chunk g: fills qT/kT[*][g], vt[*][4g..4g+3] ----
            def proj_units(g):
                n0 = g * QC
                xt = []
                for kc in range(KC):
                    t = xtp.tile([P, QC], F32, tag="xt", name="xt")
                    nc.sync.dma_start(t[:], xT_t[kc, :, n0 : n0 + QC])
                    xt.append(t)
                for which, (w_s, dst) in enumerate(((wq_s, qT), (wk_s, kT))):
                    ps = pproj.tile([P, QC], F32, tag="pp", name="pp")
                    for kc in range(KC):
                        nc.tensor.matmul(
                            ps[:], w_s[:, kc * P : (kc + 1) * P], xt[kc][:],
                            start=(kc == 0), stop=(kc == KC - 1),
                        )
                    nc.vector.tensor_copy(dst[0][g][:], ps[0:HD, :])
                    nc.vector.tensor_copy(dst[1][g][:], ps[HD:P, :])
                    yield
                for s in range(4):
                    jt = g * 4 + s
                    ps = pproj.tile([P, QC], F32, tag="pp", name="pp")
                    for kc in range(KC):
                        nc.tensor.matmul(
                            ps[:, 0:P], xt[kc][:, s * P : (s + 1) * P],
                            wv_s[:, kc * P : (kc + 1) * P],
                            start=(kc == 0), stop=(kc == KC - 1),
                        )
                    nc.vector.tensor_copy(vt[0][jt][:, 0:HD], ps[:, 0:HD])
                    nc.vector.tensor_copy(vt[1][jt][:, 0:HD], ps[:, HD:P])
                    yield

            # ---- attention for q-chunk g (proj work for g+1 interleaved) ----
            def attention(g, interleave):
                q0 = g * QC
                njt = 4 * (g + 1)
                ctx = [pctx.tile([HD + 1, QC], F32, tag="ctx", name="ctx") for h in range(2)]
                pend = None
                for jt in range(njt):
                    sc = psc.tile([P, 2 * QC], F32, tag="sc", name="sc")
                    for h in range(2):
                        nc.tensor.matmul(
                            sc[:, h * QC : (h + 1) * QC],
                            kT[h][jt // 4][:, (jt % 4) * P : (jt % 4 + 1) * P],
                            qT[h][g][:], start=True, stop=True,
                        )
                    wt = wtp.tile([P, 2 * QC], F32, tag="wt", name="wt")
                    nc.scalar.activation(wt[:], sc[:], EXP, scale=0.125)
                    if jt >= 4 * g:  # diagonal: zero where q_global < j_global
                        for h in range(2):
                            nc.gpsimd.affine_select(
                                out=wt[:, h * QC : (h + 1) * QC],
                                in_=wt[:, h * QC : (h + 1) * QC],
                                pattern=[[1, QC]], compare_op=IS_GE, fill=0.0,
                                base=q0 - jt * P, channel_multiplier=-1,
                            )
                    if pend is not None:
                        _av(*pend)
                    pend = (ctx, wt, jt, njt)
                    if interleave is not None:
                        next(interleave, None)
                _av(*pend)
                if interleave is not None:
                    for _ in interleave:
                        pass
                return ctx

            def _av(ctx, wt, jt, njt):
                for h in range(2):
                    nc.tensor.matmul(
                        ctx[h][:], vt[h][jt][:], wt[:, h * QC : (h + 1) * QC],
                        start=(jt == 0), stop=(jt == njt - 1),
                    )

            # ---- normalize + output projection for q-chunk g ----
            def finish(g, ctx):
                q0 = g * QC
                ctxs = ctxsp.tile([P, QC], F32, tag="ctxs", name="ctxs")
                for h in range(2):
                    rec = small.tile([1, QC], F32, tag="rec", name="rec")
                    nc.vector.tensor_copy(rec[:], ctx[h][HD : HD + 1, :])
                    nc.vector.reciprocal(rec[:], rec[:])
                    bc = pproj.tile([HD, QC], F32, tag="pp", name="pp")
                    nc.tensor.matmul(bc[:], ones64[:], rec[:], start=True, stop=True)
                    bcs = small.tile([HD, QC], F32, tag="bcs", name="bcs")
                    nc.vector.tensor_copy(bcs[:], bc[:])
                    nc.vector.tensor_tensor(
                        out=ctxs[h * HD : (h + 1) * HD, :],
                        in0=ctx[h][0:HD, :], in1=bcs[:], op=MULT,
                    )
                for s in range(4):
                    for oc in range(2):
                        po = pproj.tile([P, QC], F32, tag="pp", name="pp")
                        nc.tensor.matmul(
                            po[:], ctxs[:, s * P : (s + 1) * P],
                            wo_s[:, oc * QC : (oc + 1) * QC],
                            start=True, stop=True,
                        )
                        ob = outsp.tile([P, QC], F32, tag="ob", name="ob")
                        nc.vector.tensor_copy(ob[:], po[:])
                        nc.sync.dma_start(
                            out[q0 + s * P : q0 + (s + 1) * P, oc * QC : (oc + 1) * QC],
                            ob[:],
                        )

            # ---- schedule: proj(0); then attn(g) with proj(g+1) interleaved ----
            for _ in proj_units(0):
                pass
            for g in range(G):
                inter = proj_units(g + 1) if g + 1 < G else None
                ctx = attention(g, inter)
                finish(g, ctx)

    nc.compile()
    return nc


def _get_nc():
    if "nc" not in _CACHE:
        _CACHE["nc"] = build()
    return _CACHE["nc"]


def _get_runner():
    """jit(shard_map(bass_exec)) over 8 cores, built once and cached.

    Mirrors bass2jax.run_bass_via_pjrt's multi-core path minus donation, so
    the pre-zeroed output operands stay valid and every call after the first
    reuses the compiled executable.
    """
    if "runner" in _CACHE:
        return _CACHE["runner"]
    from concourse import bass2jax

    nc = _get_nc()
    bass2jax.install_neuronx_cc_hook()
    partition_name = nc.partition_id_tensor.name if nc.partition_id_tensor else None
    in_names, out_names, out_avals, zero_outs = [], [], [], []
    for alloc in nc.m.functions[0].allocations:
        if not isinstance(alloc, mybir.MemoryLocationSet):
            continue
        name = alloc.memorylocations[0].name
        if alloc.kind == "ExternalInput":
            if name != partition_name:
                in_names.append(name)
        elif alloc.kind == "ExternalOutput":
            shape = tuple(alloc.tensor_shape)
            dtype = mybir.dt.np(alloc.dtype)
            out_names.append(name)
            out_avals.append(jax.core.ShapedArray(shape, dtype))
            zero_outs.append(np.zeros(shape, dtype))
    n_params = len(in_names)
    all_in = in_names + out_names
    if partition_name is not None:
        all_in.append(partition_name)

    def _body(*args):
        operands = list(args)
        if partition_name is not None:
            operands.append(bass2jax.partition_id_tensor())
        return tuple(
            bass2jax._bass_exec_p.bind(
                *operands,
                out_avals=tuple(out_avals),
                in_names=tuple(all_in),
                out_names=tuple(out_names),
                lowering_input_output_aliases=(),
                sim_require_finite=True,
                sim_require_nnan=True,
                nc=nc,
            )
        )

    mesh = Mesh(np.asarray(jax.devices()[:8]), ("core",))
    spec = PartitionSpec("core")
    fn = jax.jit(
        shard_map(
            _body,
            mesh=mesh,
            in_specs=(spec,) * (n_params + len(out_names)),
            out_specs=(spec,) * len(out_names),
            check_rep=False,
        ),
        keep_unused=True,
    )
    sharding = NamedSharding(mesh, spec)
    zeros_dev = [
        jax.device_put(np.concatenate([z] * 8, axis=0), sharding) for z in zero_outs
    ]
    _CACHE["runner"] = (fn, in_names, out_names, out_avals, zeros_dev, sharding)
    return _CACHE["runner"]


def run_sharded(maps):
    """Run the SPMD kernel on 8 cores; returns list of per-core output dicts."""
    fn, in_names, out_names, out_avals, zeros_dev, sharding = _get_runner()
    concat_in = [
        jax.device_put(
            np.concatenate([np.asarray(maps[c][n]) for c in range(8)], axis=0), sharding
        )
        for n in in_names
    ]
    outs = fn(*concat_in, *zeros_dev)
    return [
        {
            n: np.asarray(outs[i]).reshape(8, *out_avals[i].shape)[c]
            for i, n in enumerate(out_names)
        }
        for c in range(8)
    ]


def make_in_maps(x, Wq, Wk, Wv, Wo):
    x = np.asarray(x, dtype=np.float32)
    xT = np.ascontiguousarray(x.reshape(N, D).T)
    maps = []
    for c in range(8):
        sl = slice(c * P, (c + 1) * P)

        def _w(W):
            # [1024, 128] slice -> [p, kc*128+d] so SBUF gets lhsT chunks directly
            Wc = np.asarray(W, dtype=np.float32)[:, sl]
            return np.ascontiguousarray(
                Wc.reshape(KC, P, P).transpose(1, 0, 2).reshape(P, D)
            )

        maps.append(
            {
                "xT": xT,
                "wq": _w(Wq),
                "wk": _w(Wk),
                "wv": _w(Wv),
                "wo": np.ascontiguousarray(np.asarray(Wo, dtype=np.float32)[sl, :]),
            }
        )
    return maps


def kernel(x, Wq, Wk, Wv, Wo, bo):
    maps = make_in_maps(x, Wq, Wk, Wv, Wo)
    res = run_sharded(maps)
    acc = res[0]["out"].astype(np.float32)
    for c in range(1, 8):
        acc = acc + res[c]["out"]
    acc = acc + np.asarray(bo, dtype=np.float32)[None, :]
    return acc[None].astype(np.float32)
